# revision 42
# baseline (speedup 1.0000x reference)
"""Trainium2 Bass kernel for ComputeNodeAreaFromRouteMap (DREAMPlace-style
weighted-overlap map sampling).

area_i = sum_{a,b} ovx[i,a] * ovy[i,b] * U[bx0_i+a, by0_i+b]

Strategy (gather-free): the per-node window lookup is the bottleneck on
TRN2 — the SWDGE dma_gather ucode costs ~2.5 ns/index engine-serially
(max 1024 idx/call), a ~330 us floor for 1M nodes.  Instead the host
BUCKETS nodes by their (qx2, by0) = (bx0>>1, by0) window record and
makes record identity STRUCTURAL: each of the 131072 records owns ONE
node slot per core, laid out so SBUF partition p and column c give
record r = p*1024 + c.  A record's nodes are dealt round-robin across
the 8 cores (capacity 8 nodes/record); the ~1.7% of nodes in hotter
records go to a small overflow tier whose 24-byte records the host
embeds directly in the input stream.  Empty slots hold size-0 dummies
whose clamp-difference weights vanish.

Device work per core is then pure static-AP dense math over
131072 + 20480 slots, no per-node indirection at all:
  - window table WT[r] = U[2*qx2 : 2*qx2+4, by0 : by0+3] * BSX*BSY
    (b-major 3x4 fp16 record: since node_size < 2*bin strictly, the
    5th x-tap and 4th y-tap are identically zero), streamed
    sequentially, record r at [partition r>>10, cols (r&1023)*12).
  - weights: the host pre-shifts positions into the slot's window
    frame (exact f32 affine shift), so inputs are fp16 and
    fl = pos/bs, fh = fl + size/bs; tap weights ov[t] =
    relu(min(fh,t+1) - max(fl,t)).  No floor() on device: bucketing
    already fixed the window base, and out-of-window taps auto-zero.
  - reduce: m = T*ovx (broadcast over y-taps, x-taps innermost),
    pairwise-tree sum the 4 x-taps, multiply by ovy, fold the 3
    y-taps.  Tree adds with the even-sized x-dim innermost (instead of
    tensor_reduce / an odd innermost) keep the DVE in its 2x fp16 mode.
Data-parallel over slots across the 8 NeuronCores; the table is
replicated.  Host work is bucketing/permutation and table layout; all
floating-point area math runs on device.
"""
import numpy as np

import concourse.bacc as bacc
import concourse.bass as bass
import concourse.tile as tile
import concourse.mybir as mybir
from concourse import bass_utils

# ---- problem constants (hardcoded per the task contract) ----
XL, YL, XH, YH = 0.0, 0.0, 1000.0, 1000.0
NUM_MOVABLE = 1_000_000
NBX, NBY = 512, 512
BSX = (XH - XL) / NBX            # 1.953125
BSY = (YH - YL) / NBY
INV_BSX = 1.0 / BSX
INV_BSY = 1.0 / BSY

NCORES = 8
P = 128                          # partitions
NPP = 1024                       # main slot columns per partition
NPC = P * NPP                    # 131072 main slots (= records) per core
NREC = NPC                       # records: (bx0>>1) * 512 + by0
NTAPX = 4                        # x taps 0..3 (record a-dim); tap 4 is
NTAPY = 3                        # always zero since node_size < 2*bin, as
ESIZE = NTAPX * NTAPY            # is y tap 3 (by0-granular records).
                                 # 12 fp16 elems per record (b-major)
NCHUNK = 4                       # column chunks per pass
CC = NPP // NCHUNK               # 256 slot cols per chunk
OVC = 160                        # overflow slot columns per partition
NOV = P * OVC                    # 20480 overflow slots per core
NPPO = NPP + OVC                 # output columns per partition

f32 = mybir.dt.float32
f16 = mybir.dt.float16

AL = mybir.AluOpType
AX = mybir.AxisListType


def build(repeat=1, num_cores=NCORES):
    nc = bacc.Bacc(None, target_bir_lowering=False, debug=False)

    x_in = nc.dram_tensor("x_in", [NPC], f16, kind="ExternalInput")
    y_in = nc.dram_tensor("y_in", [NPC], f16, kind="ExternalInput")
    sx_in = nc.dram_tensor("sx_in", [NPC], f16, kind="ExternalInput")
    sy_in = nc.dram_tensor("sy_in", [NPC], f16, kind="ExternalInput")
    wt_in = nc.dram_tensor("wt_in", [NREC * ESIZE], f16, kind="ExternalInput")
    ov_in = nc.dram_tensor("ov_in", [P * OVC * 4], f16, kind="ExternalInput")
    orec_in = nc.dram_tensor("orec_in", [P * OVC * ESIZE], f16,
                             kind="ExternalInput")
    area_out = nc.dram_tensor("area_out", [P * NPPO], f32,
                              kind="ExternalOutput")

    x_t = x_in[:].rearrange("(p c) -> p c", p=P)
    y_t = y_in[:].rearrange("(p c) -> p c", p=P)
    sx_t = sx_in[:].rearrange("(p c) -> p c", p=P)
    sy_t = sy_in[:].rearrange("(p c) -> p c", p=P)
    wt_t = wt_in[:].rearrange("(p c) -> p c", p=P)
    ov_t = ov_in[:].rearrange("(p c) -> p c", p=P)
    orec_t = orec_in[:].rearrange("(p c) -> p c", p=P)
    out_t = area_out[:].rearrange("(p c) -> p c", p=P)

    with tile.TileContext(nc) as tc:
        with (
            tc.tile_pool(name="const", bufs=1) as cpool,
            tc.tile_pool(name="inp", bufs=2) as xpool,
            tc.tile_pool(name="scr", bufs=1) as spool,
            tc.tile_pool(name="per", bufs=2) as gpool,
            tc.tile_pool(name="wts", bufs=2) as wpool,
            tc.tile_pool(name="tbl", bufs=2) as tpool,
            tc.tile_pool(name="red", bufs=1) as rpool,
            tc.tile_pool(name="out", bufs=2) as opool,
        ):
            iotax = cpool.tile([P, NTAPX + 1], f16)
            for k in range(NTAPX + 1):
                nc.vector.memset(iotax[:, k:k + 1], float(k))

            def weights(v, fl, fh, ncols, ntap, tag):
                """ov[t] = relu(min(fh,t+1) - max(fl,t)): [P,ncols,ntap]
                Per-tap tensor_scalar slices stay in the DVE 2x_2p mode
                (a broadcast tensor_tensor min/max would run at 1x)."""
                ov = wpool.tile([P, ncols, ntap], f16, tag=f"{tag}ov")
                d2 = spool.tile([P, ncols, ntap], f16, tag=f"{tag}d2")
                for t in range(ntap):
                    v.tensor_scalar(ov[:, :, t:t + 1], fh.unsqueeze(2),
                                    float(t + 1), None, AL.min)
                    v.tensor_scalar(d2[:, :, t:t + 1], fl.unsqueeze(2),
                                    float(t), None, AL.max)
                v.tensor_sub(ov[:], ov[:], d2[:])
                v.tensor_scalar(ov[:], ov[:], 0.0, None, AL.max)
                return ov

            def weights_x(v, fl, fh, ncols, tag):
                """x-axis: fl in [0,2) (2-bin records) so max(fl,t) = t for
                t >= 2, and fh < 4 so the t=3 cap never binds:
                  ovx[0] = relu(min(fh,1) - fl)
                  ovx[1] = relu(min(fh,2) - max(fl,1))
                  ovx[2] = clamp01(fh-2)
                  ovx[3] = relu(fh-3)"""
                ov = wpool.tile([P, ncols, NTAPX], f16, tag=f"{tag}ov")
                d2 = spool.tile([P, ncols, 2], f16, tag=f"{tag}d2")
                v.tensor_scalar(ov[:, :, 0:1], fh.unsqueeze(2), 1.0, None,
                                AL.min)
                v.tensor_scalar(ov[:, :, 1:2], fh.unsqueeze(2), 2.0, None,
                                AL.min)
                v.tensor_scalar(d2[:, :, 0:1], fl.unsqueeze(2), 0.0, None,
                                AL.max)
                v.tensor_scalar(d2[:, :, 1:2], fl.unsqueeze(2), 1.0, None,
                                AL.max)
                v.tensor_tensor(ov[:, :, 0:2], ov[:, :, 0:2], d2[:],
                                AL.subtract)
                v.tensor_scalar(ov[:, :, 0:2], ov[:, :, 0:2], 0.0, None,
                                AL.max)
                v.tensor_scalar(ov[:, :, 2:3], fh.unsqueeze(2), 2.0, 0.0,
                                AL.subtract, AL.max)
                v.tensor_scalar(ov[:, :, 2:3], ov[:, :, 2:3], 1.0, None,
                                AL.min)
                v.tensor_scalar(ov[:, :, 3:4], fh.unsqueeze(2), 3.0, 0.0,
                                AL.subtract, AL.max)
                return ov

            def weights_y(v, fl, fh, ncols, tag):
                """y-axis special case: fl in [0,1) (by0-granular records)
                so max(fl,t) = t for t >= 1, and fh < 3 so the t=2 cap
                never binds:
                  ovy[0] = min(fh,1) - fl   (>= 0 since fh >= fl)
                  ovy[1] = clamp01(fh-1)
                  ovy[2] = relu(fh-2)"""
                ov = wpool.tile([P, ncols, NTAPY], f16, tag=f"{tag}ov")
                v.tensor_scalar(ov[:, :, 0:1], fh.unsqueeze(2), 1.0, None,
                                AL.min)
                v.tensor_tensor(ov[:, :, 0:1], ov[:, :, 0:1],
                                fl.unsqueeze(2), AL.subtract)
                v.tensor_scalar(ov[:, :, 1:2], fh.unsqueeze(2), 1.0, 0.0,
                                AL.subtract, AL.max)
                v.tensor_scalar(ov[:, :, 1:2], ov[:, :, 1:2], 1.0, None,
                                AL.min)
                v.tensor_scalar(ov[:, :, 2:3], fh.unsqueeze(2), 2.0, 0.0,
                                AL.subtract, AL.max)
                return ov

            def reduce_unit(v, t4, ovx, ovy, ncols, area_ap, rtag):
                """area = sum_ab T[.,b,a] * ovx[a] * ovy[b] per slot col.
                Records are b-major so the a-dim (4 taps) is innermost:
                the ovx broadcast and the first tree level stay in the
                DVE 2x fp16 mode; only the cheap 3-tap y fold runs 1x."""
                m = rpool.tile([P, ncols, NTAPY, NTAPX], f16, tag=f"{rtag}m")
                s1 = rpool.tile([P, ncols, NTAPY, 2], f16, tag=f"{rtag}s1")
                t2 = rpool.tile([P, ncols, NTAPY], f16, tag=f"{rtag}t2")
                u1 = rpool.tile([P, ncols, 1], f16, tag=f"{rtag}u1")
                ovx_b = ovx[:].unsqueeze(2).to_broadcast(
                    [P, ncols, NTAPY, NTAPX])
                v.tensor_tensor(m[:], t4, ovx_b, AL.mult)
                v.tensor_tensor(s1[:], m[:, :, :, 0:2], m[:, :, :, 2:4],
                                AL.add)
                v.tensor_tensor(t2[:].unsqueeze(3), s1[:, :, :, 0:1],
                                s1[:, :, :, 1:2], AL.add)
                v.tensor_tensor(t2[:], t2[:], ovy[:], AL.mult)
                v.tensor_tensor(u1[:], t2[:, :, 0:1], t2[:, :, 1:2], AL.add)
                v.tensor_tensor(area_ap, u1[:], t2[:, :, 2:3], AL.add)

            def body():
                v = nc.vector
                x = xpool.tile([P, NPP], f16, tag="x")
                y = xpool.tile([P, NPP], f16, tag="y")
                sx = xpool.tile([P, NPP], f16, tag="sx")
                sy = xpool.tile([P, NPP], f16, tag="sy")
                nc.sync.dma_start(x[:], x_t)
                nc.sync.dma_start(y[:], y_t)
                nc.sync.dma_start(sx[:], sx_t)
                nc.sync.dma_start(sy[:], sy_t)

                def axis_prep(pos, size, inv_bs, tag, n=NPP):
                    """fl = pos/bs, fh = fl + size/bs (fp16; positions are
                    host-shifted into the slot's window frame)."""
                    fl = gpool.tile([P, n], f16, tag=f"{tag}fl")
                    fh = gpool.tile([P, n], f16, tag=f"{tag}fh")
                    v.tensor_scalar(fl[:], pos, inv_bs, None, AL.mult)
                    v.scalar_tensor_tensor(out=fh[:], in0=size,
                                           scalar=inv_bs, in1=fl[:],
                                           op0=AL.mult, op1=AL.add)
                    return fl, fh

                flx, fhx = axis_prep(x[:], sx[:], INV_BSX, "x")
                fly, fhy = axis_prep(y[:], sy[:], INV_BSY, "y")

                area = opool.tile([P, NPPO], f32, tag="area")
                for ch in range(NCHUNK):
                    tch = tpool.tile([P, CC * ESIZE], f16, tag="t")
                    nc.sync.dma_start(
                        tch[:], wt_t[:, ch * CC * ESIZE:
                                     (ch + 1) * CC * ESIZE])
                    t4 = tch[:].rearrange("p (c b a) -> p c b a", b=NTAPY,
                                          a=NTAPX)
                    cs = slice(ch * CC, (ch + 1) * CC)
                    ovx = weights(v, flx[:, cs], fhx[:, cs], CC, NTAPX, "wx")
                    ovy = weights(v, fly[:, cs], fhy[:, cs], CC, NTAPY, "wy")
                    a_ap = area[:, cs].unsqueeze(2)
                    reduce_unit(v, t4, ovx, ovy, CC, a_ap, "c")

                # ---- overflow tier: host-embedded records ----
                ovin = xpool.tile([P, OVC * 4], f16, tag="ovin")
                orec = xpool.tile([P, OVC * ESIZE], f16, tag="orec")
                nc.sync.dma_start(ovin[:], ov_t)
                nc.sync.dma_start(orec[:], orec_t)
                ox = ovin[:, 0 * OVC:1 * OVC]
                oy = ovin[:, 1 * OVC:2 * OVC]
                osx = ovin[:, 2 * OVC:3 * OVC]
                osy = ovin[:, 3 * OVC:4 * OVC]
                flo, fho = axis_prep(ox, osx, INV_BSX, "ox", n=OVC)
                flo2, fho2 = axis_prep(oy, osy, INV_BSY, "oy", n=OVC)
                ovxo = weights_x(v, flo[:], fho[:], OVC, "ox")
                ovyo = weights_y(v, flo2[:], fho2[:], OVC, "oy")
                r4 = orec[:].rearrange("p (c b a) -> p c b a", b=NTAPY,
                                       a=NTAPX)
                reduce_unit(v, r4, ovxo, ovyo, OVC,
                            area[:, NPP:NPPO].unsqueeze(2), "o")

                nc.sync.dma_start(out_t, area[:])

            if repeat == 1:
                body()
            else:
                with tc.For_i(0, repeat, 1, staggered_reset=True):
                    body()

    nc.compile()
    return nc


def make_table(utilization_map):
    """WT[r, a, b] = U[2*(r>>9... see layout] * BSX*BSY, fp16, a-major.
    Record r = qx2*512 + by0: rows 2*qx2 + a (a in 0..4), cols by0 + b
    (b in 0..3); map edges zero-padded."""
    U = np.asarray(utilization_map, np.float32) * np.float32(BSX * BSY)
    Upad = np.zeros((512 + NTAPX, 512 + NTAPY), np.float32)
    Upad[:512, :512] = U
    qx2 = np.arange(256)
    by0 = np.arange(512)
    a = np.arange(NTAPX)
    b = np.arange(NTAPY)
    rows = 2 * qx2[:, None, None, None] + a[None, None, None, :]
    cols = by0[None, :, None, None] + b[None, None, :, None]
    win = Upad[rows, cols]                       # [256, 512, 3(b), 4(a)]
    return win.astype(np.float16).reshape(NREC, ESIZE)


def prepare(pos, node_size_x, node_size_y, utilization_map):
    """Bucket nodes into (core, output slot); return per-core input maps
    plus each node's (core, flat output index) for unsharding."""
    n = NUM_MOVABLE
    half = pos.shape[0] // 2
    x = np.asarray(pos[:n], np.float32)
    y = np.asarray(pos[half:half + n], np.float32)
    sx = np.asarray(node_size_x, np.float32)
    sy = np.asarray(node_size_y, np.float32)

    # window base per node, matching the reference's f32 chain
    bx0 = np.clip(np.floor(x / np.float32(BSX)).astype(np.int32), 0, NBX - 1)
    by0 = np.clip(np.floor(y / np.float32(BSY)).astype(np.int32), 0, NBY - 1)
    rec = (bx0 >> 1).astype(np.int64) * 512 + by0

    order = np.argsort(rec, kind="stable")
    rs = rec[order]
    starts = np.flatnonzero(np.r_[True, np.diff(rs) != 0])
    run_id = np.cumsum(np.r_[0, (np.diff(rs) != 0).astype(np.int64)])
    pos_in_rec = np.arange(n, dtype=np.int64) - starts[run_id]
    core = pos_in_rec % NCORES
    k = pos_in_rec // NCORES
    # overflow nodes carry their record explicitly, so their core choice is
    # free — deal them globally round-robin for balance (per-record dealing
    # would pile them all onto low cores: pos 8 -> core 0, 9 -> 1, ...)
    ovsel = k >= 1
    core[ovsel] = np.arange(int(ovsel.sum()), dtype=np.int64) % NCORES

    wt2d = make_table(utilization_map)           # [NREC, 20] fp16
    # shift positions into the slot's window frame (exact f32 affine shift:
    # the window corner coordinates are exactly representable) so positions
    # ship as fp16 and the device needs no base maps
    qx2 = (bx0 >> 1).astype(np.float32)
    xl = x - qx2 * np.float32(2.0 * BSX)
    yl = y - by0.astype(np.float32) * np.float32(BSY)

    main = k < 1
    node_core = np.empty(n, np.int64)
    node_out = np.empty(n, np.int64)             # flat output index
    node_core[order] = core
    slot = rs                                    # main slot id == record id
    node_out[order[main]] = ((slot[main] // NPP) * NPPO + slot[main] % NPP)

    in_maps = []
    for c in range(NCORES):
        mc = core == c
        mcm = mc & main
        s = slot[mcm]
        idx = order[mcm]
        xp = np.zeros(NPC, np.float16)
        yp = np.zeros(NPC, np.float16)
        sxp = np.zeros(NPC, np.float16)
        syp = np.zeros(NPC, np.float16)
        xp[s] = xl[idx]
        yp[s] = yl[idx]
        sxp[s] = sx[idx]
        syp[s] = sy[idx]

        # overflow tier
        mco = mc & ~main
        oidx = order[mco]
        nov = oidx.size
        assert nov <= NOV, f"overflow {nov} exceeds capacity {NOV}"
        ovr = rs[mco]
        ovp = np.zeros((4, P, OVC), np.float16)
        orec = np.zeros((P, OVC, ESIZE), np.float16)
        op_ = np.arange(nov) // OVC
        oc_ = np.arange(nov) % OVC
        ovp[0, op_, oc_] = xl[oidx]
        ovp[1, op_, oc_] = yl[oidx]
        ovp[2, op_, oc_] = sx[oidx]
        ovp[3, op_, oc_] = sy[oidx]
        orec[op_, oc_] = wt2d[ovr]
        node_out[oidx] = op_ * NPPO + NPP + oc_

        in_maps.append(dict(
            x_in=xp, y_in=yp, sx_in=sxp, sy_in=syp,
            wt_in=wt2d.reshape(-1),
            ov_in=ovp.transpose(1, 0, 2).reshape(-1),
            orec_in=orec.reshape(-1)))
    return in_maps, (node_core, node_out)


def unshard(outs, meta):
    """outs: per-core [P*NPPO] slot-area arrays -> [N] node areas."""
    node_core, node_out = meta
    stacked = np.stack([np.asarray(o).reshape(-1) for o in outs])
    return stacked[node_core, node_out].astype(np.float32)


_NC_CACHE = {}


def _get_nc(repeat=1):
    if repeat not in _NC_CACHE:
        _NC_CACHE[repeat] = build(repeat)
    return _NC_CACHE[repeat]


def kernel(pos, node_size_x, node_size_y, utilization_map):
    in_maps, meta = prepare(pos, node_size_x, node_size_y, utilization_map)
    nc = _get_nc(1)
    res = bass_utils.run_bass_kernel_spmd(nc, in_maps,
                                          core_ids=list(range(NCORES)))
    return unshard([r["area_out"] for r in res.results], meta)


# revision 44
# speedup vs baseline: 1.1634x; 1.1634x over previous
"""Trainium2 Bass kernel for ComputeNodeAreaFromRouteMap (DREAMPlace-style
weighted-overlap map sampling).

area_i = sum_{a,b} ovx[i,a] * ovy[i,b] * U[bx0_i+a, by0_i+b]

Strategy (gather-free): the per-node window lookup is the bottleneck on
TRN2 — the SWDGE dma_gather ucode costs ~2.5 ns/index engine-serially
(max 1024 idx/call), a ~330 us floor for 1M nodes.  Instead the host
BUCKETS nodes by their (qx2, by0) = (bx0>>1, by0) window record and
makes record identity STRUCTURAL: each of the 131072 records owns ONE
node slot per core, laid out so SBUF partition p and column c give
record r = p*1024 + c.  A record's nodes are dealt round-robin across
the 8 cores (capacity 8 nodes/record); the ~1.7% of nodes in hotter
records go to a small overflow tier whose 24-byte records the host
embeds directly in the input stream.  Empty slots hold size-0 dummies
whose clamp-difference weights vanish.

Device work per core is then pure static-AP dense math over
131072 + 20480 slots, no per-node indirection at all:
  - window table WT[r] = U[2*qx2 : 2*qx2+4, by0 : by0+3] * BSX*BSY
    (b-major 3x4 fp16 record: since node_size < 2*bin strictly, the
    5th x-tap and 4th y-tap are identically zero), streamed
    sequentially, record r at [partition r>>10, cols (r&1023)*12).
  - weights: the host pre-shifts positions into the slot's window
    frame (exact f32 affine shift), so inputs are fp16 and
    fl = pos/bs, fh = fl + size/bs; tap weights ov[t] =
    relu(min(fh,t+1) - max(fl,t)).  No floor() on device: bucketing
    already fixed the window base, and out-of-window taps auto-zero.
  - reduce: m = T*ovx (broadcast over y-taps, x-taps innermost),
    pairwise-tree sum the 4 x-taps, multiply by ovy, fold the 3
    y-taps.  Tree adds with the even-sized x-dim innermost (instead of
    tensor_reduce / an odd innermost) keep the DVE in its 2x fp16 mode.
Data-parallel over slots across the 8 NeuronCores; the table is
replicated.  Host work is bucketing/permutation and table layout; all
floating-point area math runs on device.
"""
import numpy as np

import concourse.bacc as bacc
import concourse.bass as bass
import concourse.tile as tile
import concourse.mybir as mybir
from concourse import bass_utils

# ---- problem constants (hardcoded per the task contract) ----
XL, YL, XH, YH = 0.0, 0.0, 1000.0, 1000.0
NUM_MOVABLE = 1_000_000
NBX, NBY = 512, 512
BSX = (XH - XL) / NBX            # 1.953125
BSY = (YH - YL) / NBY
INV_BSX = 1.0 / BSX
INV_BSY = 1.0 / BSY

NCORES = 8
P = 128                          # partitions
NPP = 1024                       # main slot columns per partition
NPC = P * NPP                    # 131072 main slots (= records) per core
NREC = NPC                       # records: (bx0>>1) * 512 + by0
NTAPX = 4                        # x taps 0..3 (record a-dim); tap 4 is
NTAPY = 3                        # always zero since node_size < 2*bin, as
ESIZE = NTAPX * NTAPY            # is y tap 3 (by0-granular records).
                                 # 12 fp16 elems per record (b-major)
NCHUNK = 4                       # column chunks per pass
CC = NPP // NCHUNK               # 256 slot cols per chunk
OVC = 160                        # overflow slot columns per partition
NOV = P * OVC                    # 20480 overflow slots per core
NPPO = NPP + OVC                 # output columns per partition

f32 = mybir.dt.float32
f16 = mybir.dt.float16

AL = mybir.AluOpType
AX = mybir.AxisListType


def build(repeat=1, num_cores=NCORES):
    nc = bacc.Bacc(None, target_bir_lowering=False, debug=False)

    x_in = nc.dram_tensor("x_in", [NPC], f16, kind="ExternalInput")
    y_in = nc.dram_tensor("y_in", [NPC], f16, kind="ExternalInput")
    sx_in = nc.dram_tensor("sx_in", [NPC], f16, kind="ExternalInput")
    sy_in = nc.dram_tensor("sy_in", [NPC], f16, kind="ExternalInput")
    wt_in = nc.dram_tensor("wt_in", [NREC * ESIZE], f16, kind="ExternalInput")
    ov_in = nc.dram_tensor("ov_in", [P * OVC * 4], f16, kind="ExternalInput")
    orec_in = nc.dram_tensor("orec_in", [P * OVC * ESIZE], f16,
                             kind="ExternalInput")
    area_out = nc.dram_tensor("area_out", [P * NPPO], f32,
                              kind="ExternalOutput")

    x_t = x_in[:].rearrange("(p c) -> p c", p=P)
    y_t = y_in[:].rearrange("(p c) -> p c", p=P)
    sx_t = sx_in[:].rearrange("(p c) -> p c", p=P)
    sy_t = sy_in[:].rearrange("(p c) -> p c", p=P)
    wt_t = wt_in[:].rearrange("(p c) -> p c", p=P)
    ov_t = ov_in[:].rearrange("(p c) -> p c", p=P)
    orec_t = orec_in[:].rearrange("(p c) -> p c", p=P)
    out_t = area_out[:].rearrange("(p c) -> p c", p=P)

    with tile.TileContext(nc) as tc:
        with (
            tc.tile_pool(name="const", bufs=1) as cpool,
            tc.tile_pool(name="inp", bufs=2) as xpool,
            tc.tile_pool(name="scr", bufs=1) as spool,
            tc.tile_pool(name="per", bufs=2) as gpool,
            tc.tile_pool(name="wts", bufs=2) as wpool,
            tc.tile_pool(name="tbl", bufs=2) as tpool,
            tc.tile_pool(name="red", bufs=1) as rpool,
            tc.tile_pool(name="out", bufs=2) as opool,
        ):
            iotax = cpool.tile([P, NTAPX + 1], f16)
            for k in range(NTAPX + 1):
                nc.vector.memset(iotax[:, k:k + 1], float(k))

            def weights(v, fl, fh, ncols, ntap, tag):
                """ov[t] = relu(min(fh,t+1) - max(fl,t)): [P,ncols,ntap]
                Per-tap tensor_scalar slices stay in the DVE 2x_2p mode
                (a broadcast tensor_tensor min/max would run at 1x)."""
                ov = wpool.tile([P, ncols, ntap], f16, tag=f"{tag}ov")
                d2 = spool.tile([P, ncols, ntap], f16, tag=f"{tag}d2")
                for t in range(ntap):
                    v.tensor_scalar(ov[:, :, t:t + 1], fh.unsqueeze(2),
                                    float(t + 1), None, AL.min)
                    v.tensor_scalar(d2[:, :, t:t + 1], fl.unsqueeze(2),
                                    float(t), None, AL.max)
                v.tensor_sub(ov[:], ov[:], d2[:])
                v.tensor_scalar(ov[:], ov[:], 0.0, None, AL.max)
                return ov

            def weights_y(v, fl, fh, ncols, tag):
                """y-axis special case: fl in [0,1) (by0-granular records)
                so max(fl,t) = t for t >= 1, and fh < 3 so the t=2 cap
                never binds:
                  ovy[0] = min(fh,1) - fl   (>= 0 since fh >= fl)
                  ovy[1] = clamp01(fh-1)
                  ovy[2] = relu(fh-2)"""
                ov = wpool.tile([P, ncols, NTAPY], f16, tag=f"{tag}ov")
                v.tensor_scalar(ov[:, :, 0:1], fh.unsqueeze(2), 1.0, None,
                                AL.min)
                v.tensor_tensor(ov[:, :, 0:1], ov[:, :, 0:1],
                                fl.unsqueeze(2), AL.subtract)
                v.tensor_scalar(ov[:, :, 1:2], fh.unsqueeze(2), 1.0, 0.0,
                                AL.subtract, AL.max)
                v.tensor_scalar(ov[:, :, 1:2], ov[:, :, 1:2], 1.0, None,
                                AL.min)
                v.tensor_scalar(ov[:, :, 2:3], fh.unsqueeze(2), 2.0, 0.0,
                                AL.subtract, AL.max)
                return ov

            def reduce_unit(v, t4, ovx, ovy, ncols, area_ap, rtag):
                """area = sum_ab T[.,b,a] * ovx[a] * ovy[b] per slot col.
                Records are b-major so the a-dim (4 taps) is innermost:
                the ovx broadcast and the first tree level stay in the
                DVE 2x fp16 mode; only the cheap 3-tap y fold runs 1x."""
                m = rpool.tile([P, ncols, NTAPY, NTAPX], f16, tag=f"{rtag}m")
                s1 = rpool.tile([P, ncols, NTAPY, 2], f16, tag=f"{rtag}s1")
                t2 = rpool.tile([P, ncols, NTAPY], f16, tag=f"{rtag}t2")
                u1 = rpool.tile([P, ncols, 1], f16, tag=f"{rtag}u1")
                ovx_b = ovx[:].unsqueeze(2).to_broadcast(
                    [P, ncols, NTAPY, NTAPX])
                v.tensor_tensor(m[:], t4, ovx_b, AL.mult)
                v.tensor_tensor(s1[:], m[:, :, :, 0:2], m[:, :, :, 2:4],
                                AL.add)
                v.tensor_tensor(t2[:].unsqueeze(3), s1[:, :, :, 0:1],
                                s1[:, :, :, 1:2], AL.add)
                v.tensor_tensor(t2[:], t2[:], ovy[:], AL.mult)
                v.tensor_tensor(u1[:], t2[:, :, 0:1], t2[:, :, 1:2], AL.add)
                v.tensor_tensor(area_ap, u1[:], t2[:, :, 2:3], AL.add)

            def body():
                v = nc.vector
                x = xpool.tile([P, NPP], f16, tag="x")
                y = xpool.tile([P, NPP], f16, tag="y")
                sx = xpool.tile([P, NPP], f16, tag="sx")
                sy = xpool.tile([P, NPP], f16, tag="sy")
                nc.sync.dma_start(x[:], x_t)
                nc.sync.dma_start(y[:], y_t)
                nc.sync.dma_start(sx[:], sx_t)
                nc.sync.dma_start(sy[:], sy_t)
                # overflow-tier inputs issued up front so they stream
                # during the chunk loop instead of serializing its tail
                ovin = xpool.tile([P, OVC * 4], f16, tag="ovin")
                orec = xpool.tile([P, OVC * ESIZE], f16, tag="orec")
                nc.sync.dma_start(ovin[:], ov_t)
                nc.sync.dma_start(orec[:], orec_t)

                def axis_prep(pos, size, inv_bs, tag, n=NPP):
                    """fl = pos/bs, fh = fl + size/bs (fp16; positions are
                    host-shifted into the slot's window frame)."""
                    fl = gpool.tile([P, n], f16, tag=f"{tag}fl")
                    fh = gpool.tile([P, n], f16, tag=f"{tag}fh")
                    v.tensor_scalar(fl[:], pos, inv_bs, None, AL.mult)
                    v.scalar_tensor_tensor(out=fh[:], in0=size,
                                           scalar=inv_bs, in1=fl[:],
                                           op0=AL.mult, op1=AL.add)
                    return fl, fh

                flx, fhx = axis_prep(x[:], sx[:], INV_BSX, "x")
                fly, fhy = axis_prep(y[:], sy[:], INV_BSY, "y")

                area = opool.tile([P, NPPO], f32, tag="area")
                for ch in range(NCHUNK):
                    tch = tpool.tile([P, CC * ESIZE], f16, tag="t")
                    nc.sync.dma_start(
                        tch[:], wt_t[:, ch * CC * ESIZE:
                                     (ch + 1) * CC * ESIZE])
                    t4 = tch[:].rearrange("p (c b a) -> p c b a", b=NTAPY,
                                          a=NTAPX)
                    cs = slice(ch * CC, (ch + 1) * CC)
                    ovx = weights(v, flx[:, cs], fhx[:, cs], CC, NTAPX, "wx")
                    ovy = weights(v, fly[:, cs], fhy[:, cs], CC, NTAPY, "wy")
                    a_ap = area[:, cs].unsqueeze(2)
                    reduce_unit(v, t4, ovx, ovy, CC, a_ap, "c")

                # ---- overflow tier: host-embedded records ----
                ox = ovin[:, 0 * OVC:1 * OVC]
                oy = ovin[:, 1 * OVC:2 * OVC]
                osx = ovin[:, 2 * OVC:3 * OVC]
                osy = ovin[:, 3 * OVC:4 * OVC]
                flo, fho = axis_prep(ox, osx, INV_BSX, "ox", n=OVC)
                flo2, fho2 = axis_prep(oy, osy, INV_BSY, "oy", n=OVC)
                ovxo = weights(v, flo[:], fho[:], OVC, NTAPX, "ox")
                ovyo = weights_y(v, flo2[:], fho2[:], OVC, "oy")
                r4 = orec[:].rearrange("p (c b a) -> p c b a", b=NTAPY,
                                       a=NTAPX)
                reduce_unit(v, r4, ovxo, ovyo, OVC,
                            area[:, NPP:NPPO].unsqueeze(2), "o")

                nc.sync.dma_start(out_t, area[:])

            if repeat == 1:
                body()
            else:
                with tc.For_i(0, repeat, 1, staggered_reset=True):
                    body()

    nc.compile()
    return nc


def make_table(utilization_map):
    """WT[r, a, b] = U[2*(r>>9... see layout] * BSX*BSY, fp16, a-major.
    Record r = qx2*512 + by0: rows 2*qx2 + a (a in 0..4), cols by0 + b
    (b in 0..3); map edges zero-padded."""
    U = np.asarray(utilization_map, np.float32) * np.float32(BSX * BSY)
    Upad = np.zeros((512 + NTAPX, 512 + NTAPY), np.float32)
    Upad[:512, :512] = U
    qx2 = np.arange(256)
    by0 = np.arange(512)
    a = np.arange(NTAPX)
    b = np.arange(NTAPY)
    rows = 2 * qx2[:, None, None, None] + a[None, None, None, :]
    cols = by0[None, :, None, None] + b[None, None, :, None]
    win = Upad[rows, cols]                       # [256, 512, 3(b), 4(a)]
    return win.astype(np.float16).reshape(NREC, ESIZE)


def prepare(pos, node_size_x, node_size_y, utilization_map):
    """Bucket nodes into (core, output slot); return per-core input maps
    plus each node's (core, flat output index) for unsharding."""
    n = NUM_MOVABLE
    half = pos.shape[0] // 2
    x = np.asarray(pos[:n], np.float32)
    y = np.asarray(pos[half:half + n], np.float32)
    sx = np.asarray(node_size_x, np.float32)
    sy = np.asarray(node_size_y, np.float32)

    # window base per node, matching the reference's f32 chain
    bx0 = np.clip(np.floor(x / np.float32(BSX)).astype(np.int32), 0, NBX - 1)
    by0 = np.clip(np.floor(y / np.float32(BSY)).astype(np.int32), 0, NBY - 1)
    rec = (bx0 >> 1).astype(np.int64) * 512 + by0

    order = np.argsort(rec, kind="stable")
    rs = rec[order]
    starts = np.flatnonzero(np.r_[True, np.diff(rs) != 0])
    run_id = np.cumsum(np.r_[0, (np.diff(rs) != 0).astype(np.int64)])
    pos_in_rec = np.arange(n, dtype=np.int64) - starts[run_id]
    core = pos_in_rec % NCORES
    k = pos_in_rec // NCORES
    # overflow nodes carry their record explicitly, so their core choice is
    # free — deal them globally round-robin for balance (per-record dealing
    # would pile them all onto low cores: pos 8 -> core 0, 9 -> 1, ...)
    ovsel = k >= 1
    core[ovsel] = np.arange(int(ovsel.sum()), dtype=np.int64) % NCORES

    wt2d = make_table(utilization_map)           # [NREC, 20] fp16
    # shift positions into the slot's window frame (exact f32 affine shift:
    # the window corner coordinates are exactly representable) so positions
    # ship as fp16 and the device needs no base maps
    qx2 = (bx0 >> 1).astype(np.float32)
    xl = x - qx2 * np.float32(2.0 * BSX)
    yl = y - by0.astype(np.float32) * np.float32(BSY)

    main = k < 1
    node_core = np.empty(n, np.int64)
    node_out = np.empty(n, np.int64)             # flat output index
    node_core[order] = core
    slot = rs                                    # main slot id == record id
    node_out[order[main]] = ((slot[main] // NPP) * NPPO + slot[main] % NPP)

    in_maps = []
    for c in range(NCORES):
        mc = core == c
        mcm = mc & main
        s = slot[mcm]
        idx = order[mcm]
        xp = np.zeros(NPC, np.float16)
        yp = np.zeros(NPC, np.float16)
        sxp = np.zeros(NPC, np.float16)
        syp = np.zeros(NPC, np.float16)
        xp[s] = xl[idx]
        yp[s] = yl[idx]
        sxp[s] = sx[idx]
        syp[s] = sy[idx]

        # overflow tier
        mco = mc & ~main
        oidx = order[mco]
        nov = oidx.size
        assert nov <= NOV, f"overflow {nov} exceeds capacity {NOV}"
        ovr = rs[mco]
        ovp = np.zeros((4, P, OVC), np.float16)
        orec = np.zeros((P, OVC, ESIZE), np.float16)
        op_ = np.arange(nov) // OVC
        oc_ = np.arange(nov) % OVC
        ovp[0, op_, oc_] = xl[oidx]
        ovp[1, op_, oc_] = yl[oidx]
        ovp[2, op_, oc_] = sx[oidx]
        ovp[3, op_, oc_] = sy[oidx]
        orec[op_, oc_] = wt2d[ovr]
        node_out[oidx] = op_ * NPPO + NPP + oc_

        in_maps.append(dict(
            x_in=xp, y_in=yp, sx_in=sxp, sy_in=syp,
            wt_in=wt2d.reshape(-1),
            ov_in=ovp.transpose(1, 0, 2).reshape(-1),
            orec_in=orec.reshape(-1)))
    return in_maps, (node_core, node_out)


def unshard(outs, meta):
    """outs: per-core [P*NPPO] slot-area arrays -> [N] node areas."""
    node_core, node_out = meta
    stacked = np.stack([np.asarray(o).reshape(-1) for o in outs])
    return stacked[node_core, node_out].astype(np.float32)


_NC_CACHE = {}


def _get_nc(repeat=1):
    if repeat not in _NC_CACHE:
        _NC_CACHE[repeat] = build(repeat)
    return _NC_CACHE[repeat]


def kernel(pos, node_size_x, node_size_y, utilization_map):
    in_maps, meta = prepare(pos, node_size_x, node_size_y, utilization_map)
    nc = _get_nc(1)
    res = bass_utils.run_bass_kernel_spmd(nc, in_maps,
                                          core_ids=list(range(NCORES)))
    return unshard([r["area_out"] for r in res.results], meta)


# revision 45
# speedup vs baseline: 1.2033x; 1.0343x over previous
"""Trainium2 Bass kernel for ComputeNodeAreaFromRouteMap (DREAMPlace-style
weighted-overlap map sampling).

area_i = sum_{a,b} ovx[i,a] * ovy[i,b] * U[bx0_i+a, by0_i+b]

Strategy (gather-free): the per-node window lookup is the bottleneck on
TRN2 — the SWDGE dma_gather ucode costs ~2.5 ns/index engine-serially
(max 1024 idx/call), a ~330 us floor for 1M nodes.  Instead the host
BUCKETS nodes by their (qx2, by0) = (bx0>>1, by0) window record and
makes record identity STRUCTURAL: each of the 131072 records owns ONE
node slot per core, laid out so SBUF partition p and column c give
record r = p*1024 + c.  A record's nodes are dealt round-robin across
the 8 cores (capacity 8 nodes/record); the ~1.7% of nodes in hotter
records go to a small overflow tier whose 24-byte records the host
embeds directly in the input stream.  Empty slots hold size-0 dummies
whose clamp-difference weights vanish.

Device work per core is then pure static-AP dense math over
131072 + 20480 slots, no per-node indirection at all:
  - window table WT[r] = U[2*qx2 : 2*qx2+4, by0 : by0+3] * BSX*BSY
    (b-major 3x4 fp16 record: since node_size < 2*bin strictly, the
    5th x-tap and 4th y-tap are identically zero), streamed
    sequentially, record r at [partition r>>10, cols (r&1023)*12).
  - weights: the host pre-shifts positions into the slot's window
    frame (exact f32 affine shift), so inputs are fp16 and
    fl = pos/bs, fh = fl + size/bs; tap weights ov[t] =
    relu(min(fh,t+1) - max(fl,t)).  No floor() on device: bucketing
    already fixed the window base, and out-of-window taps auto-zero.
  - reduce: m = T*ovx (broadcast over y-taps, x-taps innermost),
    pairwise-tree sum the 4 x-taps, multiply by ovy, fold the 3
    y-taps.  Tree adds with the even-sized x-dim innermost (instead of
    tensor_reduce / an odd innermost) keep the DVE in its 2x fp16 mode.
Data-parallel over slots across the 8 NeuronCores; the table is
replicated.  Host work is bucketing/permutation and table layout; all
floating-point area math runs on device.
"""
import numpy as np

import concourse.bacc as bacc
import concourse.bass as bass
import concourse.tile as tile
import concourse.mybir as mybir
from concourse import bass_utils

# ---- problem constants (hardcoded per the task contract) ----
XL, YL, XH, YH = 0.0, 0.0, 1000.0, 1000.0
NUM_MOVABLE = 1_000_000
NBX, NBY = 512, 512
BSX = (XH - XL) / NBX            # 1.953125
BSY = (YH - YL) / NBY
INV_BSX = 1.0 / BSX
INV_BSY = 1.0 / BSY

NCORES = 8
P = 128                          # partitions
NPP = 1024                       # main slot columns per partition
NPC = P * NPP                    # 131072 main slots (= records) per core
NREC = NPC                       # records: (bx0>>1) * 512 + by0
NTAPX = 4                        # x taps 0..3 (record a-dim); tap 4 is
NTAPY = 3                        # always zero since node_size < 2*bin, as
ESIZE = NTAPX * NTAPY            # is y tap 3 (by0-granular records).
                                 # 12 fp16 elems per record (b-major)
NCHUNK = 4                       # column chunks per pass
CC = NPP // NCHUNK               # 256 slot cols per chunk
OVC = 160                        # overflow slot columns per partition
NOV = P * OVC                    # 20480 overflow slots per core
NPPO = NPP + OVC                 # output columns per partition

f32 = mybir.dt.float32
f16 = mybir.dt.float16

AL = mybir.AluOpType
AX = mybir.AxisListType


def build(repeat=1, num_cores=NCORES):
    nc = bacc.Bacc(None, target_bir_lowering=False, debug=False)

    x_in = nc.dram_tensor("x_in", [NPC], f16, kind="ExternalInput")
    y_in = nc.dram_tensor("y_in", [NPC], f16, kind="ExternalInput")
    sx_in = nc.dram_tensor("sx_in", [NPC], f16, kind="ExternalInput")
    sy_in = nc.dram_tensor("sy_in", [NPC], f16, kind="ExternalInput")
    wt_in = nc.dram_tensor("wt_in", [NREC * ESIZE], f16, kind="ExternalInput")
    ov_in = nc.dram_tensor("ov_in", [P * OVC * 4], f16, kind="ExternalInput")
    orec_in = nc.dram_tensor("orec_in", [P * OVC * ESIZE], f16,
                             kind="ExternalInput")
    area_out = nc.dram_tensor("area_out", [P * NPPO], f32,
                              kind="ExternalOutput")

    x_t = x_in[:].rearrange("(p c) -> p c", p=P)
    y_t = y_in[:].rearrange("(p c) -> p c", p=P)
    sx_t = sx_in[:].rearrange("(p c) -> p c", p=P)
    sy_t = sy_in[:].rearrange("(p c) -> p c", p=P)
    wt_t = wt_in[:].rearrange("(p c) -> p c", p=P)
    ov_t = ov_in[:].rearrange("(p c) -> p c", p=P)
    orec_t = orec_in[:].rearrange("(p c) -> p c", p=P)
    out_t = area_out[:].rearrange("(p c) -> p c", p=P)

    with tile.TileContext(nc) as tc:
        with (
            tc.tile_pool(name="const", bufs=1) as cpool,
            tc.tile_pool(name="inp", bufs=2) as xpool,
            tc.tile_pool(name="scr", bufs=1) as spool,
            tc.tile_pool(name="per", bufs=2) as gpool,
            tc.tile_pool(name="wts", bufs=2) as wpool,
            tc.tile_pool(name="tbl", bufs=2) as tpool,
            tc.tile_pool(name="red", bufs=1) as rpool,
            tc.tile_pool(name="out", bufs=2) as opool,
        ):
            iotax = cpool.tile([P, NTAPX + 1], f16)
            for k in range(NTAPX + 1):
                nc.vector.memset(iotax[:, k:k + 1], float(k))

            def weights(v, fl, fh, ncols, ntap, tag):
                """ov[t] = relu(min(fh,t+1) - max(fl,t)): [P,ncols,ntap]
                Per-tap tensor_scalar slices stay in the DVE 2x_2p mode
                (a broadcast tensor_tensor min/max would run at 1x)."""
                ov = wpool.tile([P, ncols, ntap], f16, tag=f"{tag}ov")
                d2 = spool.tile([P, ncols, ntap], f16, tag=f"{tag}d2")
                for t in range(ntap):
                    v.tensor_scalar(ov[:, :, t:t + 1], fh.unsqueeze(2),
                                    float(t + 1), None, AL.min)
                    v.tensor_scalar(d2[:, :, t:t + 1], fl.unsqueeze(2),
                                    float(t), None, AL.max)
                v.tensor_sub(ov[:], ov[:], d2[:])
                v.tensor_scalar(ov[:], ov[:], 0.0, None, AL.max)
                return ov

            def weights_y(v, fl, fh, ncols, tag):
                """y-axis special case: fl in [0,1) (by0-granular records)
                so max(fl,t) = t for t >= 1, and fh < 3 so the t=2 cap
                never binds:
                  ovy[0] = min(fh,1) - fl   (>= 0 since fh >= fl)
                  ovy[1] = clamp01(fh-1)
                  ovy[2] = relu(fh-2)"""
                ov = wpool.tile([P, ncols, NTAPY], f16, tag=f"{tag}ov")
                v.tensor_scalar(ov[:, :, 0:1], fh.unsqueeze(2), 1.0, None,
                                AL.min)
                v.tensor_tensor(ov[:, :, 0:1], ov[:, :, 0:1],
                                fl.unsqueeze(2), AL.subtract)
                v.tensor_scalar(ov[:, :, 1:2], fh.unsqueeze(2), 1.0, 0.0,
                                AL.subtract, AL.max)
                v.tensor_scalar(ov[:, :, 1:2], ov[:, :, 1:2], 1.0, None,
                                AL.min)
                v.tensor_scalar(ov[:, :, 2:3], fh.unsqueeze(2), 2.0, 0.0,
                                AL.subtract, AL.max)
                return ov

            def reduce_unit(v, t4, ovx, ovy, ncols, area_ap, rtag):
                """area = sum_ab T[.,b,a] * ovx[a] * ovy[b] per slot col.
                Records are b-major so the a-dim (4 taps) is innermost:
                the ovx broadcast and the first tree level stay in the
                DVE 2x fp16 mode; only the cheap 3-tap y fold runs 1x."""
                m = rpool.tile([P, ncols, NTAPY, NTAPX], f16, tag=f"{rtag}m")
                s1 = rpool.tile([P, ncols, NTAPY, 2], f16, tag=f"{rtag}s1")
                t2 = rpool.tile([P, ncols, NTAPY], f16, tag=f"{rtag}t2")
                u1 = rpool.tile([P, ncols, 1], f16, tag=f"{rtag}u1")
                ovx_b = ovx[:].unsqueeze(2).to_broadcast(
                    [P, ncols, NTAPY, NTAPX])
                v.tensor_tensor(m[:], t4, ovx_b, AL.mult)
                v.tensor_tensor(s1[:], m[:, :, :, 0:2], m[:, :, :, 2:4],
                                AL.add)
                v.tensor_tensor(t2[:].unsqueeze(3), s1[:, :, :, 0:1],
                                s1[:, :, :, 1:2], AL.add)
                v.tensor_tensor(t2[:], t2[:], ovy[:], AL.mult)
                v.tensor_tensor(u1[:], t2[:, :, 0:1], t2[:, :, 1:2], AL.add)
                v.tensor_tensor(area_ap, u1[:], t2[:, :, 2:3], AL.add)

            def body():
                v = nc.vector
                x = xpool.tile([P, NPP], f16, tag="x")
                y = xpool.tile([P, NPP], f16, tag="y")
                sx = xpool.tile([P, NPP], f16, tag="sx")
                sy = xpool.tile([P, NPP], f16, tag="sy")
                nc.sync.dma_start(x[:], x_t)
                nc.sync.dma_start(y[:], y_t)
                nc.sync.dma_start(sx[:], sx_t)
                nc.sync.dma_start(sy[:], sy_t)
                # overflow-tier inputs issued up front so they stream
                # during the chunk loop instead of serializing its tail
                ovin = xpool.tile([P, OVC * 4], f16, tag="ovin")
                orec = xpool.tile([P, OVC * ESIZE], f16, tag="orec")
                nc.sync.dma_start(ovin[:], ov_t)
                nc.sync.dma_start(orec[:], orec_t)

                def axis_prep(pos, size, inv_bs, tag, n=NPP):
                    """fl = pos/bs, fh = fl + size/bs (fp16; positions are
                    host-shifted into the slot's window frame)."""
                    fl = gpool.tile([P, n], f16, tag=f"{tag}fl")
                    fh = gpool.tile([P, n], f16, tag=f"{tag}fh")
                    v.tensor_scalar(fl[:], pos, inv_bs, None, AL.mult)
                    v.scalar_tensor_tensor(out=fh[:], in0=size,
                                           scalar=inv_bs, in1=fl[:],
                                           op0=AL.mult, op1=AL.add)
                    return fl, fh

                flx, fhx = axis_prep(x[:], sx[:], INV_BSX, "x")
                fly, fhy = axis_prep(y[:], sy[:], INV_BSY, "y")

                area = opool.tile([P, NPPO], f32, tag="area")
                # issue every chunk's table DMA up front (distinct tags)
                # so the whole table streams during the chunk loop
                tchs = []
                for ch in range(NCHUNK):
                    tch = tpool.tile([P, CC * ESIZE], f16, tag=f"t{ch}")
                    nc.sync.dma_start(
                        tch[:], wt_t[:, ch * CC * ESIZE:
                                     (ch + 1) * CC * ESIZE])
                    tchs.append(tch)
                for ch in range(NCHUNK):
                    t4 = tchs[ch][:].rearrange("p (c b a) -> p c b a",
                                               b=NTAPY, a=NTAPX)
                    cs = slice(ch * CC, (ch + 1) * CC)
                    ovx = weights(v, flx[:, cs], fhx[:, cs], CC, NTAPX, "wx")
                    ovy = weights(v, fly[:, cs], fhy[:, cs], CC, NTAPY, "wy")
                    a_ap = area[:, cs].unsqueeze(2)
                    reduce_unit(v, t4, ovx, ovy, CC, a_ap, "c")

                # ---- overflow tier: host-embedded records ----
                ox = ovin[:, 0 * OVC:1 * OVC]
                oy = ovin[:, 1 * OVC:2 * OVC]
                osx = ovin[:, 2 * OVC:3 * OVC]
                osy = ovin[:, 3 * OVC:4 * OVC]
                flo, fho = axis_prep(ox, osx, INV_BSX, "ox", n=OVC)
                flo2, fho2 = axis_prep(oy, osy, INV_BSY, "oy", n=OVC)
                ovxo = weights(v, flo[:], fho[:], OVC, NTAPX, "ox")
                ovyo = weights_y(v, flo2[:], fho2[:], OVC, "oy")
                r4 = orec[:].rearrange("p (c b a) -> p c b a", b=NTAPY,
                                       a=NTAPX)
                reduce_unit(v, r4, ovxo, ovyo, OVC,
                            area[:, NPP:NPPO].unsqueeze(2), "o")

                nc.sync.dma_start(out_t, area[:])

            if repeat == 1:
                body()
            else:
                with tc.For_i(0, repeat, 1, staggered_reset=True):
                    body()

    nc.compile()
    return nc


def make_table(utilization_map):
    """WT[r, a, b] = U[2*(r>>9... see layout] * BSX*BSY, fp16, a-major.
    Record r = qx2*512 + by0: rows 2*qx2 + a (a in 0..4), cols by0 + b
    (b in 0..3); map edges zero-padded."""
    U = np.asarray(utilization_map, np.float32) * np.float32(BSX * BSY)
    Upad = np.zeros((512 + NTAPX, 512 + NTAPY), np.float32)
    Upad[:512, :512] = U
    qx2 = np.arange(256)
    by0 = np.arange(512)
    a = np.arange(NTAPX)
    b = np.arange(NTAPY)
    rows = 2 * qx2[:, None, None, None] + a[None, None, None, :]
    cols = by0[None, :, None, None] + b[None, None, :, None]
    win = Upad[rows, cols]                       # [256, 512, 3(b), 4(a)]
    return win.astype(np.float16).reshape(NREC, ESIZE)


def prepare(pos, node_size_x, node_size_y, utilization_map):
    """Bucket nodes into (core, output slot); return per-core input maps
    plus each node's (core, flat output index) for unsharding."""
    n = NUM_MOVABLE
    half = pos.shape[0] // 2
    x = np.asarray(pos[:n], np.float32)
    y = np.asarray(pos[half:half + n], np.float32)
    sx = np.asarray(node_size_x, np.float32)
    sy = np.asarray(node_size_y, np.float32)

    # window base per node, matching the reference's f32 chain
    bx0 = np.clip(np.floor(x / np.float32(BSX)).astype(np.int32), 0, NBX - 1)
    by0 = np.clip(np.floor(y / np.float32(BSY)).astype(np.int32), 0, NBY - 1)
    rec = (bx0 >> 1).astype(np.int64) * 512 + by0

    order = np.argsort(rec, kind="stable")
    rs = rec[order]
    starts = np.flatnonzero(np.r_[True, np.diff(rs) != 0])
    run_id = np.cumsum(np.r_[0, (np.diff(rs) != 0).astype(np.int64)])
    pos_in_rec = np.arange(n, dtype=np.int64) - starts[run_id]
    core = pos_in_rec % NCORES
    k = pos_in_rec // NCORES
    # overflow nodes carry their record explicitly, so their core choice is
    # free — deal them globally round-robin for balance (per-record dealing
    # would pile them all onto low cores: pos 8 -> core 0, 9 -> 1, ...)
    ovsel = k >= 1
    core[ovsel] = np.arange(int(ovsel.sum()), dtype=np.int64) % NCORES

    wt2d = make_table(utilization_map)           # [NREC, 20] fp16
    # shift positions into the slot's window frame (exact f32 affine shift:
    # the window corner coordinates are exactly representable) so positions
    # ship as fp16 and the device needs no base maps
    qx2 = (bx0 >> 1).astype(np.float32)
    xl = x - qx2 * np.float32(2.0 * BSX)
    yl = y - by0.astype(np.float32) * np.float32(BSY)

    main = k < 1
    node_core = np.empty(n, np.int64)
    node_out = np.empty(n, np.int64)             # flat output index
    node_core[order] = core
    slot = rs                                    # main slot id == record id
    node_out[order[main]] = ((slot[main] // NPP) * NPPO + slot[main] % NPP)

    in_maps = []
    for c in range(NCORES):
        mc = core == c
        mcm = mc & main
        s = slot[mcm]
        idx = order[mcm]
        xp = np.zeros(NPC, np.float16)
        yp = np.zeros(NPC, np.float16)
        sxp = np.zeros(NPC, np.float16)
        syp = np.zeros(NPC, np.float16)
        xp[s] = xl[idx]
        yp[s] = yl[idx]
        sxp[s] = sx[idx]
        syp[s] = sy[idx]

        # overflow tier
        mco = mc & ~main
        oidx = order[mco]
        nov = oidx.size
        assert nov <= NOV, f"overflow {nov} exceeds capacity {NOV}"
        ovr = rs[mco]
        ovp = np.zeros((4, P, OVC), np.float16)
        orec = np.zeros((P, OVC, ESIZE), np.float16)
        op_ = np.arange(nov) // OVC
        oc_ = np.arange(nov) % OVC
        ovp[0, op_, oc_] = xl[oidx]
        ovp[1, op_, oc_] = yl[oidx]
        ovp[2, op_, oc_] = sx[oidx]
        ovp[3, op_, oc_] = sy[oidx]
        orec[op_, oc_] = wt2d[ovr]
        node_out[oidx] = op_ * NPPO + NPP + oc_

        in_maps.append(dict(
            x_in=xp, y_in=yp, sx_in=sxp, sy_in=syp,
            wt_in=wt2d.reshape(-1),
            ov_in=ovp.transpose(1, 0, 2).reshape(-1),
            orec_in=orec.reshape(-1)))
    return in_maps, (node_core, node_out)


def unshard(outs, meta):
    """outs: per-core [P*NPPO] slot-area arrays -> [N] node areas."""
    node_core, node_out = meta
    stacked = np.stack([np.asarray(o).reshape(-1) for o in outs])
    return stacked[node_core, node_out].astype(np.float32)


_NC_CACHE = {}


def _get_nc(repeat=1):
    if repeat not in _NC_CACHE:
        _NC_CACHE[repeat] = build(repeat)
    return _NC_CACHE[repeat]


def kernel(pos, node_size_x, node_size_y, utilization_map):
    in_maps, meta = prepare(pos, node_size_x, node_size_y, utilization_map)
    nc = _get_nc(1)
    res = bass_utils.run_bass_kernel_spmd(nc, in_maps,
                                          core_ids=list(range(NCORES)))
    return unshard([r["area_out"] for r in res.results], meta)
